# revision 3
# baseline (speedup 1.0000x reference)
"""Multi-head attention (B=4, S=2048, D=1024, H=16, d_k=64) on 8 NeuronCores.

Sharding: core c handles batch b=c//2 and head-group g=c%2 (8 heads, 512
features). Each core computes Q/K/V projections for its head group,
attention, and a partial output projection (row-split W_o). Host sums the
two partial outputs per batch.

Device layout notes:
- Host pre-transposes activations to x^T [D, S] so the projection matmuls
  can consume them directly (contraction along partitions).
- Q and K are produced transposed ([feat, S]) so scores are computed in
  the "scores^T" layout [sk, sq]: softmax sums land on the free axis via a
  ones-row appended to V (stationary [sk, 65] -> row 64 of PSUM = sums).
  No max-subtraction is needed (|score/8| <= ~7, exp is safe in fp32).
- All matmuls run in bf16 (fp32 PSUM accumulation).
"""

import sys

sys.path.insert(0, "/opt/trn_rl_repo")

import numpy as np
import ml_dtypes

BF = ml_dtypes.bfloat16

S = 2048          # sequence length
D = 1024          # model dim
F = 512           # features per core (8 heads x 64)
HPC = 8           # heads per core
DK = 64           # head dim
P = 128           # partitions
NCORES = 8
KC = D // P       # 8 contraction chunks for projections
ST = S // P       # 16 sequence tiles of 128
SG = S // 512     # 4 sequence groups of 512
FC = F // P       # 4 feature chunks of 128


def _build_program():
    import concourse.bass as bass
    import concourse.mybir as mybir
    import concourse.tile as tile
    from concourse import bacc

    dt = mybir.dt
    f32 = dt.float32
    bf16 = dt.bfloat16
    EXP = mybir.ActivationFunctionType.Exp

    nc = bacc.Bacc("TRN2", target_bir_lowering=False, debug=False,
                   num_devices=NCORES)

    xq_d = nc.declare_dram_parameter("xq", [D, S], bf16, isOutput=False)
    xk_d = nc.declare_dram_parameter("xk", [D, S], bf16, isOutput=False)
    xv_d = nc.declare_dram_parameter("xv", [D, S], bf16, isOutput=False)
    wq_d = nc.declare_dram_parameter("wq", [D, F], bf16, isOutput=False)
    wk_d = nc.declare_dram_parameter("wk", [D, F], bf16, isOutput=False)
    wv_d = nc.declare_dram_parameter("wv", [D, F], bf16, isOutput=False)
    wo_d = nc.declare_dram_parameter("wo", [F, D], bf16, isOutput=False)
    out_d = nc.declare_dram_parameter("out", [S, D], f32, isOutput=True)

    xq_t = xq_d.ap().rearrange("(c p) s -> c p s", p=P)
    xk_t = xk_d.ap().rearrange("(c p) s -> c p s", p=P)
    xv_t = xv_d.ap().rearrange("(c p) s -> c p s", p=P)
    wq_t = wq_d.ap().rearrange("(c p) f -> c p f", p=P)
    wk_t = wk_d.ap().rearrange("(c p) f -> c p f", p=P)
    wv_t = wv_d.ap().rearrange("(c p) f -> c p f", p=P)
    wo_t = wo_d.ap().rearrange("(c p) o -> c p o", p=P)
    out_t = out_d.ap().rearrange("(t p) o -> t p o", p=P)

    with tile.TileContext(nc) as tc:
        with (
            tc.tile_pool(name="wpool", bufs=1) as wpool,
            tc.tile_pool(name="xpool", bufs=10) as xpool,
            tc.tile_pool(name="qkpool", bufs=1) as qkpool,
            tc.tile_pool(name="vpool", bufs=1) as vpool,
            tc.tile_pool(name="apool", bufs=8) as apool,
            tc.tile_pool(name="cpool", bufs=1) as cpool,
            tc.tile_pool(name="opool", bufs=4) as opool,
            tc.tile_pool(name="spool", bufs=4) as spool,
            tc.tile_pool(name="mmps", bufs=4, space="PSUM") as mmps,
            tc.tile_pool(name="pvps", bufs=3, space="PSUM") as pvps,
            tc.tile_pool(name="bcps", bufs=1, space="PSUM") as bcps,
        ):
            # ---- load weights ----
            w_sb = {}
            for nm, src in (("wq", wq_t), ("wk", wk_t), ("wv", wv_t)):
                for c in range(KC):
                    t = wpool.tile([P, F], bf16, tag=f"{nm}{c}", name=f"{nm}{c}")
                    nc.sync.dma_start(t[:], src[c])
                    w_sb[nm, c] = t
            wo_sb = []
            for c in range(FC):
                t = wpool.tile([P, D], bf16, tag=f"wo{c}", name=f"wo{c}")
                nc.sync.dma_start(t[:], wo_t[c])
                wo_sb.append(t)

            # ones row lives at partition 64 to align with the PV sums row
            ones_sb = spool.tile([DK + 1, DK], f32, tag="ones", bufs=1,
                                 name="ones")
            nc.gpsimd.memset(ones_sb[:], 1.0)

            # ---- Q^T / K^T projections: out [F, S] as 4 tiles [128, S] ----
            qt_sb = [qkpool.tile([P, S], bf16, tag=f"qt{i}", name=f"qt{i}")
                     for i in range(FC)]
            kt_sb = [qkpool.tile([P, S], bf16, tag=f"kt{i}", name=f"kt{i}")
                     for i in range(FC)]
            for nm, src, dsts in (("wq", xq_t, qt_sb), ("wk", xk_t, kt_sb)):
                x_sb = []
                for c in range(KC):
                    xt = xpool.tile([P, S], bf16, tag="xt", name=f"x_{nm}{c}")
                    nc.sync.dma_start(xt[:], src[c])
                    x_sb.append(xt)
                for sg in range(SG):
                    for fc in range(FC):
                        ps = mmps.tile([P, 512], f32, tag="mm", name="ps_proj")
                        for c in range(KC):
                            nc.tensor.matmul(
                                ps[:],
                                w_sb[nm, c][:, fc * P:(fc + 1) * P],
                                x_sb[c][:, sg * 512:(sg + 1) * 512],
                                start=(c == 0), stop=(c == KC - 1),
                            )
                        nc.vector.tensor_copy(
                            dsts[fc][:, sg * 512:(sg + 1) * 512], ps[:])

            # ---- V projection: normal layout, per-head ones column ----
            # v_sb[t] is [128, 8, 65]: per head 64 features + a ones column.
            v_sb = []
            xv_sb = []
            for c in range(KC):
                xt = xpool.tile([P, S], bf16, tag="xt", name=f"x_v{c}")
                nc.sync.dma_start(xt[:], xv_t[c])
                xv_sb.append(xt)
            for t in range(ST):
                ps = mmps.tile([P, 512], f32, tag="mm", name="ps_v")
                for c in range(KC):
                    nc.tensor.matmul(
                        ps[:],
                        xv_sb[c][:, t * P:(t + 1) * P],
                        w_sb["wv", c][:],
                        start=(c == 0), stop=(c == KC - 1),
                    )
                vt = vpool.tile([P, HPC, DK + 1], bf16, tag=f"v{t}",
                                name=f"v{t}")
                nc.gpsimd.memset(vt[:], 1.0)
                nc.vector.tensor_copy(
                    vt[:, :, 0:DK],
                    ps.rearrange("p (h d) -> p h d", h=HPC))
                v_sb.append(vt)

            # ---- attention ----
            ctx_sb = [cpool.tile([P, S], bf16, tag=f"ctx{i}", name=f"ctx{i}")
                      for i in range(FC)]
            for hp in range(FC):          # head pair -> one qt/kt/ctx tile
                for sg in range(SG):
                    cps = [pvps.tile([DK + 1, 512], f32, tag="pv",
                                     name="ps_ctx") for _ in range(2)]
                    for sk in range(ST):
                        for h2 in range(2):
                            hq = slice(h2 * DK, (h2 + 1) * DK)
                            sps = mmps.tile([P, 512], f32, tag="mm",
                                            name="ps_qk")
                            nc.tensor.matmul(
                                sps[:],
                                kt_sb[hp][hq, sk * P:(sk + 1) * P],
                                qt_sb[hp][hq, sg * 512:(sg + 1) * 512],
                                start=True, stop=True,
                            )
                            at = apool.tile([P, 512], bf16, tag="attn",
                                            name="attn")
                            nc.scalar.activation(at[:], sps[:], EXP,
                                                 scale=0.125)
                            nc.tensor.matmul(
                                cps[h2][:],
                                v_sb[sk][:, hp * 2 + h2, :],
                                at[:],
                                start=(sk == 0), stop=(sk == ST - 1),
                            )
                    for h2 in range(2):
                        # 1/sums: DVE reads the PSUM sums row in place
                        # (partition 64), writes SBUF at the same partition.
                        rin = spool.tile([DK + 1, 512], f32, tag="rin",
                                         name="rin")
                        nc.vector.reciprocal(rin[DK:DK + 1, :],
                                             cps[h2][DK:DK + 1, :])
                        # broadcast over partitions 0..63 via K=1 matmul
                        bc = bcps.tile([DK, 512], f32, tag="bc", name="ps_bc")
                        nc.tensor.matmul(bc[:], ones_sb[DK:DK + 1, :],
                                         rin[DK:DK + 1, :],
                                         start=True, stop=True)
                        bcs = opool.tile([DK, 512], f32, tag="bcs",
                                         name="bcs")
                        nc.vector.tensor_copy(bcs[:], bc[:])
                        if h2 == 0:
                            nc.vector.tensor_mul(
                                ctx_sb[hp][0:DK, sg * 512:(sg + 1) * 512],
                                cps[h2][0:DK, :], bcs[:])
                        else:
                            tmp = opool.tile([DK, 512], bf16, tag="ctmp",
                                             name="ctmp")
                            nc.vector.tensor_mul(tmp[:], cps[h2][0:DK, :],
                                                 bcs[:])
                            nc.sync.dma_start(
                                ctx_sb[hp][DK:P, sg * 512:(sg + 1) * 512],
                                tmp[:])

            # ---- output projection: out[s, :] partial ----
            for t in range(ST):
                for og in range(2):
                    ps = mmps.tile([P, 512], f32, tag="mm", name="ps_out")
                    for fc in range(FC):
                        nc.tensor.matmul(
                            ps[:],
                            ctx_sb[fc][:, t * P:(t + 1) * P],
                            wo_sb[fc][:, og * 512:(og + 1) * 512],
                            start=(fc == 0), stop=(fc == FC - 1),
                        )
                    ot = opool.tile([P, 512], f32, tag="out", name="out_sb")
                    nc.vector.tensor_copy(ot[:], ps[:])
                    nc.sync.dma_start(out_t[t][:, og * 512:(og + 1) * 512],
                                      ot[:])

    nc.compile()
    return nc


_NC_CACHE = None


def _get_program():
    global _NC_CACHE
    if _NC_CACHE is None:
        _NC_CACHE = _build_program()
    return _NC_CACHE


def kernel(q, k, v, W_q, W_k, W_v, W_o):
    from concourse.bass_utils import run_bass_kernel_spmd

    q = np.asarray(q, np.float32)
    k = np.asarray(k, np.float32)
    v = np.asarray(v, np.float32)
    W_q = np.asarray(W_q, np.float32)
    W_k = np.asarray(W_k, np.float32)
    W_v = np.asarray(W_v, np.float32)
    W_o = np.asarray(W_o, np.float32)

    nc = _get_program()
    in_maps = []
    for c in range(NCORES):
        b, g = c // 2, c % 2
        sl = slice(g * F, (g + 1) * F)
        in_maps.append({
            "xq": np.ascontiguousarray(q[b].T).astype(BF),
            "xk": np.ascontiguousarray(k[b].T).astype(BF),
            "xv": np.ascontiguousarray(v[b].T).astype(BF),
            "wq": np.ascontiguousarray(W_q[sl, :].T).astype(BF),
            "wk": np.ascontiguousarray(W_k[sl, :].T).astype(BF),
            "wv": np.ascontiguousarray(W_v[sl, :].T).astype(BF),
            "wo": np.ascontiguousarray(W_o[:, sl].T).astype(BF),
        })
    res = run_bass_kernel_spmd(nc, in_maps, list(range(NCORES)))
    outs = [res.results[c]["out"] for c in range(NCORES)]
    full = np.stack([outs[2 * b] + outs[2 * b + 1] for b in range(4)])
    return full.astype(np.float32)


# revision 5
# speedup vs baseline: 33.7928x; 33.7928x over previous
"""Multi-head attention (B=4, S=2048, D=1024, H=16, d_k=64) on 8 NeuronCores.

Sharding: core c handles batch b=c//2 and head-group g=c%2 (8 heads, 512
features). Each core computes Q/K/V projections for its head group,
attention, and a partial output projection (row-split W_o). Host sums the
two partial outputs per batch.

Device layout notes:
- Host pre-transposes activations to x^T [D, S] so the projection matmuls
  can consume them directly (contraction along partitions).
- Q and K are produced transposed ([feat, S]) so scores are computed in
  the "scores^T" layout [sk, sq]: softmax sums land on the free axis via a
  ones-row appended to V (stationary [sk, 65] -> row 64 of PSUM = sums).
  No max-subtraction is needed (|score/8| <= ~7, exp is safe in fp32).
- All matmuls run in bf16 (fp32 PSUM accumulation).
"""

import sys

sys.path.insert(0, "/opt/trn_rl_repo")

import numpy as np
import ml_dtypes

BF = ml_dtypes.bfloat16

S = 2048          # sequence length
D = 1024          # model dim
F = 512           # features per core (8 heads x 64)
HPC = 8           # heads per core
DK = 64           # head dim
P = 128           # partitions
NCORES = 8
KC = D // P       # 8 contraction chunks for projections
ST = S // P       # 16 sequence tiles of 128
SG = S // 512     # 4 sequence groups of 512
FC = F // P       # 4 feature chunks of 128


def _build_program(reps=1):
    import concourse.bass as bass
    import concourse.mybir as mybir
    import concourse.tile as tile
    from concourse import bacc

    dt = mybir.dt
    f32 = dt.float32
    bf16 = dt.bfloat16
    EXP = mybir.ActivationFunctionType.Exp

    nc = bacc.Bacc("TRN2", target_bir_lowering=False, debug=False,
                   num_devices=NCORES)

    xq_d = nc.declare_dram_parameter("xq", [D, S], bf16, isOutput=False)
    xk_d = nc.declare_dram_parameter("xk", [D, S], bf16, isOutput=False)
    xv_d = nc.declare_dram_parameter("xv", [D, S], bf16, isOutput=False)
    wq_d = nc.declare_dram_parameter("wq", [D, F], bf16, isOutput=False)
    wk_d = nc.declare_dram_parameter("wk", [D, F], bf16, isOutput=False)
    wv_d = nc.declare_dram_parameter("wv", [D, F], bf16, isOutput=False)
    wo_d = nc.declare_dram_parameter("wo", [F, D], bf16, isOutput=False)
    out_d = nc.declare_dram_parameter("out", [S, D], f32, isOutput=True)

    xq_t = xq_d.ap().rearrange("(c p) s -> c p s", p=P)
    xk_t = xk_d.ap().rearrange("(c p) s -> c p s", p=P)
    xv_t = xv_d.ap().rearrange("(c p) s -> c p s", p=P)
    wq_t = wq_d.ap().rearrange("(c p) f -> c p f", p=P)
    wk_t = wk_d.ap().rearrange("(c p) f -> c p f", p=P)
    wv_t = wv_d.ap().rearrange("(c p) f -> c p f", p=P)
    wo_t = wo_d.ap().rearrange("(c p) o -> c p o", p=P)
    out_t = out_d.ap().rearrange("(t p) o -> t p o", p=P)

    with tile.TileContext(nc) as tc:
      for rep in range(reps):
        with (
            tc.tile_pool(name=f"wpool{rep}", bufs=1) as wpool,
            tc.tile_pool(name=f"xpool{rep}", bufs=10) as xpool,
            tc.tile_pool(name=f"qkpool{rep}", bufs=1) as qkpool,
            tc.tile_pool(name=f"vpool{rep}", bufs=1) as vpool,
            tc.tile_pool(name=f"apool{rep}", bufs=8) as apool,
            tc.tile_pool(name=f"cpool{rep}", bufs=1) as cpool,
            tc.tile_pool(name=f"opool{rep}", bufs=4) as opool,
            tc.tile_pool(name=f"spool{rep}", bufs=4) as spool,
            tc.tile_pool(name=f"mmps{rep}", bufs=4, space="PSUM") as mmps,
            tc.tile_pool(name=f"pvps{rep}", bufs=3, space="PSUM") as pvps,
            tc.tile_pool(name=f"bcps{rep}", bufs=1, space="PSUM") as bcps,
        ):
            # ---- load weights ----
            w_sb = {}
            for nm, src in (("wq", wq_t), ("wk", wk_t), ("wv", wv_t)):
                for c in range(KC):
                    t = wpool.tile([P, F], bf16, tag=f"{nm}{c}", name=f"{nm}{c}")
                    nc.sync.dma_start(t[:], src[c])
                    w_sb[nm, c] = t
            wo_sb = []
            for c in range(FC):
                t = wpool.tile([P, D], bf16, tag=f"wo{c}", name=f"wo{c}")
                nc.sync.dma_start(t[:], wo_t[c])
                wo_sb.append(t)

            # ones row lives at partition 64 to align with the PV sums row
            ones_sb = spool.tile([DK + 1, DK], f32, tag="ones", bufs=1,
                                 name="ones")
            nc.gpsimd.memset(ones_sb[:], 1.0)

            # ---- Q^T / K^T projections: out [F, S] as 4 tiles [128, S] ----
            qt_sb = [qkpool.tile([P, S], bf16, tag=f"qt{i}", name=f"qt{i}")
                     for i in range(FC)]
            kt_sb = [qkpool.tile([P, S], bf16, tag=f"kt{i}", name=f"kt{i}")
                     for i in range(FC)]
            for nm, src, dsts in (("wq", xq_t, qt_sb), ("wk", xk_t, kt_sb)):
                x_sb = []
                for c in range(KC):
                    xt = xpool.tile([P, S], bf16, tag="xt", name=f"x_{nm}{c}")
                    nc.sync.dma_start(xt[:], src[c])
                    x_sb.append(xt)
                for sg in range(SG):
                    for fc in range(FC):
                        ps = mmps.tile([P, 512], f32, tag="mm", name="ps_proj")
                        for c in range(KC):
                            nc.tensor.matmul(
                                ps[:],
                                w_sb[nm, c][:, fc * P:(fc + 1) * P],
                                x_sb[c][:, sg * 512:(sg + 1) * 512],
                                start=(c == 0), stop=(c == KC - 1),
                            )
                        nc.vector.tensor_copy(
                            dsts[fc][:, sg * 512:(sg + 1) * 512], ps[:])

            # ---- V projection: normal layout, per-head ones column ----
            # v_sb[t] is [128, 8, 65]: per head 64 features + a ones column.
            v_sb = []
            xv_sb = []
            for c in range(KC):
                xt = xpool.tile([P, S], bf16, tag="xt", name=f"x_v{c}")
                nc.sync.dma_start(xt[:], xv_t[c])
                xv_sb.append(xt)
            for t in range(ST):
                ps = mmps.tile([P, 512], f32, tag="mm", name="ps_v")
                for c in range(KC):
                    nc.tensor.matmul(
                        ps[:],
                        xv_sb[c][:, t * P:(t + 1) * P],
                        w_sb["wv", c][:],
                        start=(c == 0), stop=(c == KC - 1),
                    )
                vt = vpool.tile([P, HPC, DK + 1], bf16, tag=f"v{t}",
                                name=f"v{t}")
                nc.gpsimd.memset(vt[:], 1.0)
                nc.vector.tensor_copy(
                    vt[:, :, 0:DK],
                    ps.rearrange("p (h d) -> p h d", h=HPC))
                v_sb.append(vt)

            # ---- attention ----
            ctx_sb = [cpool.tile([P, S], bf16, tag=f"ctx{i}", name=f"ctx{i}")
                      for i in range(FC)]
            for hp in range(FC):          # head pair -> one qt/kt/ctx tile
                for sg in range(SG):
                    cps = [pvps.tile([DK + 1, 512], f32, tag="pv",
                                     name="ps_ctx") for _ in range(2)]
                    for sk in range(ST):
                        for h2 in range(2):
                            hq = slice(h2 * DK, (h2 + 1) * DK)
                            sps = mmps.tile([P, 512], f32, tag="mm",
                                            name="ps_qk")
                            nc.tensor.matmul(
                                sps[:],
                                kt_sb[hp][hq, sk * P:(sk + 1) * P],
                                qt_sb[hp][hq, sg * 512:(sg + 1) * 512],
                                start=True, stop=True,
                            )
                            at = apool.tile([P, 512], bf16, tag="attn",
                                            name="attn")
                            nc.scalar.activation(at[:], sps[:], EXP,
                                                 scale=0.125)
                            nc.tensor.matmul(
                                cps[h2][:],
                                v_sb[sk][:, hp * 2 + h2, :],
                                at[:],
                                start=(sk == 0), stop=(sk == ST - 1),
                            )
                    for h2 in range(2):
                        # 1/sums: DVE reads the PSUM sums row in place
                        # (partition 64), writes SBUF at the same partition.
                        rin = spool.tile([DK + 1, 512], f32, tag="rin",
                                         name="rin")
                        nc.vector.reciprocal(rin[DK:DK + 1, :],
                                             cps[h2][DK:DK + 1, :])
                        # broadcast over partitions 0..63 via K=1 matmul
                        bc = bcps.tile([DK, 512], f32, tag="bc", name="ps_bc")
                        nc.tensor.matmul(bc[:], ones_sb[DK:DK + 1, :],
                                         rin[DK:DK + 1, :],
                                         start=True, stop=True)
                        bcs = opool.tile([DK, 512], f32, tag="bcs",
                                         name="bcs")
                        nc.vector.tensor_copy(bcs[:], bc[:])
                        if h2 == 0:
                            nc.vector.tensor_mul(
                                ctx_sb[hp][0:DK, sg * 512:(sg + 1) * 512],
                                cps[h2][0:DK, :], bcs[:])
                        else:
                            tmp = opool.tile([DK, 512], bf16, tag="ctmp",
                                             name="ctmp")
                            nc.vector.tensor_mul(tmp[:], cps[h2][0:DK, :],
                                                 bcs[:])
                            nc.sync.dma_start(
                                ctx_sb[hp][DK:P, sg * 512:(sg + 1) * 512],
                                tmp[:])

            # ---- output projection: out[s, :] partial ----
            for t in range(ST):
                for og in range(2):
                    ps = mmps.tile([P, 512], f32, tag="mm", name="ps_out")
                    for fc in range(FC):
                        nc.tensor.matmul(
                            ps[:],
                            ctx_sb[fc][:, t * P:(t + 1) * P],
                            wo_sb[fc][:, og * 512:(og + 1) * 512],
                            start=(fc == 0), stop=(fc == FC - 1),
                        )
                    ot = opool.tile([P, 512], f32, tag="out", name="out_sb")
                    nc.vector.tensor_copy(ot[:], ps[:])
                    nc.sync.dma_start(out_t[t][:, og * 512:(og + 1) * 512],
                                      ot[:])

    nc.compile()
    return nc


_NC_CACHE = None


def _get_program():
    global _NC_CACHE
    if _NC_CACHE is None:
        _NC_CACHE = _build_program()
    return _NC_CACHE


def kernel(q, k, v, W_q, W_k, W_v, W_o):
    from concourse.bass_utils import run_bass_kernel_spmd

    q = np.asarray(q, np.float32)
    k = np.asarray(k, np.float32)
    v = np.asarray(v, np.float32)
    W_q = np.asarray(W_q, np.float32)
    W_k = np.asarray(W_k, np.float32)
    W_v = np.asarray(W_v, np.float32)
    W_o = np.asarray(W_o, np.float32)

    nc = _get_program()
    in_maps = []
    for c in range(NCORES):
        b, g = c // 2, c % 2
        sl = slice(g * F, (g + 1) * F)
        in_maps.append({
            "xq": np.ascontiguousarray(q[b].T).astype(BF),
            "xk": np.ascontiguousarray(k[b].T).astype(BF),
            "xv": np.ascontiguousarray(v[b].T).astype(BF),
            "wq": np.ascontiguousarray(W_q[sl, :].T).astype(BF),
            "wk": np.ascontiguousarray(W_k[sl, :].T).astype(BF),
            "wv": np.ascontiguousarray(W_v[sl, :].T).astype(BF),
            "wo": np.ascontiguousarray(W_o[:, sl].T).astype(BF),
        })
    res = run_bass_kernel_spmd(nc, in_maps, list(range(NCORES)))
    outs = [res.results[c]["out"] for c in range(NCORES)]
    full = np.stack([outs[2 * b] + outs[2 * b + 1] for b in range(4)])
    return full.astype(np.float32)


# revision 10
# speedup vs baseline: 36.7445x; 1.0873x over previous
"""Multi-head attention (B=4, S=2048, D=1024, H=16, d_k=64) on 8 NeuronCores.

Sharding: core c handles batch b=c//2 and head-group g=c%2 (8 heads, 512
features). Each core computes Q/K/V projections for its head group,
attention, and a partial output projection (row-split W_o). Host sums the
two partial outputs per batch.

Device layout notes:
- Host pre-transposes activations to x^T [D, S] so the projection matmuls
  can consume them directly (contraction along partitions).
- Q and K are produced transposed ([feat, S]) so scores are computed in
  the "scores^T" layout [sk, sq]: softmax sums land on the free axis via a
  ones-row appended to V (stationary [sk, 65] -> row 64 of PSUM = sums).
  No max-subtraction is needed (|score/8| <= ~7, exp is safe in fp32).
- All matmuls run in bf16 (fp32 PSUM accumulation).
"""

import sys

sys.path.insert(0, "/opt/trn_rl_repo")

import numpy as np
import ml_dtypes

BF = ml_dtypes.bfloat16

S = 2048          # sequence length
D = 1024          # model dim
F = 512           # features per core (8 heads x 64)
HPC = 8           # heads per core
DK = 64           # head dim
P = 128           # partitions
NCORES = 8
KC = D // P       # 8 contraction chunks for projections
ST = S // P       # 16 sequence tiles of 128
SG = S // 512     # 4 sequence groups of 512
FC = F // P       # 4 feature chunks of 128


def _build_program(reps=1):
    import concourse.bass as bass
    import concourse.mybir as mybir
    import concourse.tile as tile
    from concourse import bacc

    dt = mybir.dt
    f32 = dt.float32
    bf16 = dt.bfloat16
    EXP = mybir.ActivationFunctionType.Exp

    nc = bacc.Bacc("TRN2", target_bir_lowering=False, debug=False,
                   num_devices=NCORES)

    xq_d = nc.declare_dram_parameter("xq", [D, S], bf16, isOutput=False)
    xk_d = nc.declare_dram_parameter("xk", [D, S], bf16, isOutput=False)
    xv_d = nc.declare_dram_parameter("xv", [D, S], bf16, isOutput=False)
    wq_d = nc.declare_dram_parameter("wq", [D, F], bf16, isOutput=False)
    wk_d = nc.declare_dram_parameter("wk", [D, F], bf16, isOutput=False)
    wv_d = nc.declare_dram_parameter("wv", [D, F], bf16, isOutput=False)
    wo_d = nc.declare_dram_parameter("wo", [F, D], bf16, isOutput=False)
    out_d = nc.declare_dram_parameter("out", [S, D], f32, isOutput=True)

    xq_t = xq_d.ap().rearrange("(c p) s -> c p s", p=P)
    xk_t = xk_d.ap().rearrange("(c p) s -> c p s", p=P)
    xv_t = xv_d.ap().rearrange("(c p) s -> c p s", p=P)
    wq_t = wq_d.ap().rearrange("(c p) f -> c p f", p=P)
    wk_t = wk_d.ap().rearrange("(c p) f -> c p f", p=P)
    wv_t = wv_d.ap().rearrange("(c p) f -> c p f", p=P)
    wo_t = wo_d.ap().rearrange("(c p) o -> c p o", p=P)
    out_t = out_d.ap().rearrange("(t p) o -> t p o", p=P)

    with tile.TileContext(nc) as tc:
      for rep in range(reps):
        with (
            tc.tile_pool(name=f"wpool{rep}", bufs=1) as wpool,
            tc.tile_pool(name=f"xpool{rep}", bufs=10) as xpool,
            tc.tile_pool(name=f"qkpool{rep}", bufs=1) as qkpool,
            tc.tile_pool(name=f"vpool{rep}", bufs=1) as vpool,
            tc.tile_pool(name=f"apool{rep}", bufs=8) as apool,
            tc.tile_pool(name=f"cpool{rep}", bufs=1) as cpool,
            tc.tile_pool(name=f"opool{rep}", bufs=4) as opool,
            tc.tile_pool(name=f"spool{rep}", bufs=4) as spool,
            tc.tile_pool(name=f"mmps{rep}", bufs=2, space="PSUM") as mmps,
            tc.tile_pool(name=f"pvps{rep}", bufs=4, space="PSUM") as pvps,
        ):
            # ---- load weights ----
            w_sb = {}
            for nm, src in (("wq", wq_t), ("wk", wk_t), ("wv", wv_t)):
                for c in range(KC):
                    t = wpool.tile([P, F], bf16, tag=f"{nm}{c}", name=f"{nm}{c}")
                    nc.sync.dma_start(t[:], src[c])
                    w_sb[nm, c] = t
            wo_sb = []
            for c in range(FC):
                t = wpool.tile([P, D], bf16, tag=f"wo{c}", name=f"wo{c}")
                nc.sync.dma_start(t[:], wo_t[c])
                wo_sb.append(t)


            # ---- Q^T / K^T projections: out [F, S] as 4 tiles [128, S] ----
            qt_sb = [qkpool.tile([P, S], bf16, tag=f"qt{i}", name=f"qt{i}")
                     for i in range(FC)]
            kt_sb = [qkpool.tile([P, S], bf16, tag=f"kt{i}", name=f"kt{i}")
                     for i in range(FC)]
            for nm, src, dsts in (("wq", xq_t, qt_sb), ("wk", xk_t, kt_sb)):
                x_sb = []
                for c in range(KC):
                    xt = xpool.tile([P, S], bf16, tag="xt", name=f"x_{nm}{c}")
                    nc.sync.dma_start(xt[:], src[c])
                    x_sb.append(xt)
                for sg in range(SG):
                    for fc in range(FC):
                        ps = mmps.tile([P, 512], f32, tag="mm", name="ps_proj")
                        for c in range(KC):
                            nc.tensor.matmul(
                                ps[:],
                                w_sb[nm, c][:, fc * P:(fc + 1) * P],
                                x_sb[c][:, sg * 512:(sg + 1) * 512],
                                start=(c == 0), stop=(c == KC - 1),
                            )
                        nc.vector.tensor_copy(
                            dsts[fc][:, sg * 512:(sg + 1) * 512], ps[:])

            # ---- V projection: normal layout, per-head ones column ----
            # v_sb[t] is [128, 8, 65]: per head 64 features + a ones column.
            v_sb = []
            xv_sb = []
            for c in range(KC):
                xt = xpool.tile([P, S], bf16, tag="xt", name=f"x_v{c}")
                nc.sync.dma_start(xt[:], xv_t[c])
                xv_sb.append(xt)
            for t in range(ST):
                ps = mmps.tile([P, 512], f32, tag="mm", name="ps_v")
                for c in range(KC):
                    nc.tensor.matmul(
                        ps[:],
                        xv_sb[c][:, t * P:(t + 1) * P],
                        w_sb["wv", c][:],
                        start=(c == 0), stop=(c == KC - 1),
                    )
                vt = vpool.tile([P, HPC, DK + 1], bf16, tag=f"v{t}",
                                name=f"v{t}")
                nc.gpsimd.memset(vt[:], 1.0)
                nc.vector.tensor_copy(
                    vt[:, :, 0:DK],
                    ps.rearrange("p (h d) -> p h d", h=HPC))
                v_sb.append(vt)

            # ---- attention ----
            # Per (head-pair, sq-group-pair): QK for two sq groups lands in
            # one [128, 1024] PSUM tile (2 banks) so exp runs as one wide
            # ACT op; 4 ctx accumulators (2 heads x 2 sq groups) live in
            # the other 4 banks.
            ctx_sb = [cpool.tile([P, S], bf16, tag=f"ctx{i}", name=f"ctx{i}")
                      for i in range(FC)]
            for hp in range(FC):          # head pair -> one qt/kt/ctx tile
                for sgp in range(SG // 2):
                    sg0 = 2 * sgp
                    cps = [[pvps.tile([DK + 1, 512], f32, tag="pv",
                                      name="ps_ctx") for _ in range(2)]
                           for _ in range(2)]       # [h2][sgi]
                    for sk in range(ST):
                        sps = [mmps.tile([P, 1024], f32, tag="mm",
                                         name="ps_qk") for _ in range(2)]
                        # interleave h2 so adjacent matmuls hit different
                        # PE row groups (rows 0-63 vs 64-127) and overlap
                        for sgi in range(2):
                            for h2 in range(2):
                                hq = slice(h2 * DK, (h2 + 1) * DK)
                                nc.tensor.matmul(
                                    sps[h2][:, sgi * 512:(sgi + 1) * 512],
                                    kt_sb[hp][hq, sk * P:(sk + 1) * P],
                                    qt_sb[hp][hq,
                                              (sg0 + sgi) * 512:
                                              (sg0 + sgi + 1) * 512],
                                    start=True, stop=True,
                                )
                        ats = []
                        for h2 in range(2):
                            at = apool.tile([P, 1024], bf16, tag="attn",
                                            name="attn")
                            nc.scalar.activation(at[:], sps[h2][:], EXP,
                                                 scale=0.125)
                            ats.append(at)
                        for h2 in range(2):
                            for sgi in range(2):
                                nc.tensor.matmul(
                                    cps[h2][sgi][:],
                                    v_sb[sk][:, hp * 2 + h2, :],
                                    ats[h2][:, sgi * 512:(sgi + 1) * 512],
                                    start=(sk == 0), stop=(sk == ST - 1),
                                )
                    for h2 in range(2):
                        for sgi in range(2):
                            sg = sg0 + sgi
                            # 1/sums: DVE reads the PSUM sums row (partition
                            # 64) and writes a partition-0 tile (1-partition
                            # cross-quadrant move, HW-verified). The gpsimd
                            # broadcast ucode uses the tile's partition 0, so
                            # the source MUST live at partition 0.
                            rin = spool.tile([1, 512], f32, tag="rin",
                                             name="rin")
                            nc.vector.reciprocal(rin[0:1, :],
                                                 cps[h2][sgi][DK:DK + 1, :])
                            bcs = opool.tile([DK, 512], f32, tag="bcs",
                                             name="bcs")
                            nc.gpsimd.partition_broadcast(bcs[:],
                                                          rin[0:1, :])
                            if h2 == 0:
                                nc.vector.tensor_mul(
                                    ctx_sb[hp][0:DK, sg * 512:(sg + 1) * 512],
                                    cps[h2][sgi][0:DK, :], bcs[:])
                            else:
                                tmp = opool.tile([DK, 512], bf16, tag="ctmp",
                                                 name="ctmp")
                                nc.vector.tensor_mul(tmp[:],
                                                     cps[h2][sgi][0:DK, :],
                                                     bcs[:])
                                nc.sync.dma_start(
                                    ctx_sb[hp][DK:P,
                                               sg * 512:(sg + 1) * 512],
                                    tmp[:])

            # ---- output projection: out[s, :] partial ----
            for t in range(ST):
                for og in range(2):
                    ps = mmps.tile([P, 512], f32, tag="mm", name="ps_out")
                    for fc in range(FC):
                        nc.tensor.matmul(
                            ps[:],
                            ctx_sb[fc][:, t * P:(t + 1) * P],
                            wo_sb[fc][:, og * 512:(og + 1) * 512],
                            start=(fc == 0), stop=(fc == FC - 1),
                        )
                    ot = opool.tile([P, 512], f32, tag="out", name="out_sb")
                    nc.vector.tensor_copy(ot[:], ps[:])
                    nc.sync.dma_start(out_t[t][:, og * 512:(og + 1) * 512],
                                      ot[:])

    nc.compile()
    return nc


_NC_CACHE = None


def _get_program():
    global _NC_CACHE
    if _NC_CACHE is None:
        _NC_CACHE = _build_program()
    return _NC_CACHE


def kernel(q, k, v, W_q, W_k, W_v, W_o):
    from concourse.bass_utils import run_bass_kernel_spmd

    q = np.asarray(q, np.float32)
    k = np.asarray(k, np.float32)
    v = np.asarray(v, np.float32)
    W_q = np.asarray(W_q, np.float32)
    W_k = np.asarray(W_k, np.float32)
    W_v = np.asarray(W_v, np.float32)
    W_o = np.asarray(W_o, np.float32)

    nc = _get_program()
    in_maps = []
    for c in range(NCORES):
        b, g = c // 2, c % 2
        sl = slice(g * F, (g + 1) * F)
        in_maps.append({
            "xq": np.ascontiguousarray(q[b].T).astype(BF),
            "xk": np.ascontiguousarray(k[b].T).astype(BF),
            "xv": np.ascontiguousarray(v[b].T).astype(BF),
            "wq": np.ascontiguousarray(W_q[sl, :].T).astype(BF),
            "wk": np.ascontiguousarray(W_k[sl, :].T).astype(BF),
            "wv": np.ascontiguousarray(W_v[sl, :].T).astype(BF),
            "wo": np.ascontiguousarray(W_o[:, sl].T).astype(BF),
        })
    res = run_bass_kernel_spmd(nc, in_maps, list(range(NCORES)))
    outs = [res.results[c]["out"] for c in range(NCORES)]
    full = np.stack([outs[2 * b] + outs[2 * b + 1] for b in range(4)])
    return full.astype(np.float32)
